# revision 28
# baseline (speedup 1.0000x reference)
"""Distributed multi-head attention for 8 TRN2 NeuronCores — v4 (AllGather).

Problem: x[2,2048,1024] -> QKV proj (w_qkv[3072,1024]) -> 16-head SDPA ->
out proj (w_proj[1024,1024] + b_proj) -> [2,2048,1024].

Sharding: 2 heads per core (head-parallel; both batches on every core).
  Phase A: qT/kT [128, 4096] and V-natural from x @ w_qkv_shard.T.
  Phase B: transposed-score attention per (batch, 512-query chunk) u:
           S^T = kT.T @ qT (row-tiled K=64 pairs), P = exp(S*scale),
           O^T_ext[65,n] = [V|1].T @ P^T accumulated in PSUM (row 64 =
           denominator). Normalized o_n2[128ch, 512tok] per unit.
  Exchange: per-unit AllGather of o_n2 (128KB in, 1MB out, Shared) —
           8 small AGs overlap attention compute.
  Out-proj at destination: each core computes out rows for ITS 64-token
           slice of unit u (cc_rank dynamic slice): [64,1024] over all
           1024 channels, + bias -> fp32 out rows. Host reorders rows.
"""
import sys, os, types
import numpy as np

if "/opt/trn_rl_repo" not in sys.path and os.path.isdir("/opt/trn_rl_repo"):
    sys.path.append("/opt/trn_rl_repo")

import concourse.bass as bass
import concourse.mybir as mybir
import concourse.tile as tile
from concourse import bacc
from concourse.bass_utils import run_bass_kernel_spmd

F32 = mybir.dt.float32
F16 = mybir.dt.float16
BF16 = mybir.dt.bfloat16
EXP = mybir.ActivationFunctionType.Exp

NCORES = 8
B, N, C, H, D = 2, 2048, 1024, 16, 64
NT = B * N
KT = C // 128
QC = 512
NU = NT // QC
NMT = N // 128
SCALE = 1.0 / 8.0
GRP = 2
XCH = 1024

TRACE = False
LAST_EXEC_NS = None

_NC = None


def _install_ntff_hook():
    if "antenv.axon_hooks" in sys.modules:
        return
    try:
        import antenv
        from trn_agent_boot.trn_boot import _ntff_profile_via_ctypes
        mod = types.ModuleType("antenv.axon_hooks")
        _hook = [None]
        mod.set_axon_ntff_profile_hook = lambda h: _hook.__setitem__(0, h)
        mod.get_axon_ntff_profile_hook = lambda: _hook[0]
        sys.modules["antenv.axon_hooks"] = mod
        antenv.axon_hooks = mod
        mod.set_axon_ntff_profile_hook(
            _ntff_profile_via_ctypes("/opt/axon/libaxon_pjrt.so"))
    except Exception:
        pass


def _build():
    nc = bacc.Bacc("TRN2", target_bir_lowering=False, debug=False,
                   num_devices=NCORES)
    xT_ext = nc.dram_tensor("xT", [C, NT], BF16, kind="ExternalInput").ap()
    wT_ext = nc.dram_tensor("wT", [C, 384], BF16, kind="ExternalInput").ap()
    wpT_ext = nc.dram_tensor("wpT", [C, C], BF16, kind="ExternalInput").ap()
    bias_ext = nc.dram_tensor("bias", [1, C], F32, kind="ExternalInput").ap()
    idn_ext = nc.dram_tensor("idn", [128, 128], BF16, kind="ExternalInput").ap()
    out_ext = nc.dram_tensor("out", [NT // NCORES, C], F32,
                             kind="ExternalOutput").ap()
    ag_in = [nc.dram_tensor(f"ag_in{u}", [128, QC], BF16).ap()
             for u in range(NU)]
    ag_out = [nc.dram_tensor(f"ag_out{u}", [NCORES * 128, QC], BF16,
                             addr_space="Shared").ap()
              for u in range(NU)]

    xT_v = xT_ext.rearrange("(kt p) n -> p kt n", p=128)
    wT_v = wT_ext.rearrange("(kt p) f -> p kt f", p=128)
    wpT_v = wpT_ext.rearrange("(kt p) f -> p kt f", p=128)

    with tile.TileContext(nc) as tc:
        rank = nc.sync.cc_rank([list(range(NCORES))])
        with (
            tc.tile_pool(name="const", bufs=1) as cpool,
            tc.tile_pool(name="resid", bufs=1) as rpool,
        ):
            wT_sb = cpool.tile([128, KT, 384], BF16)
            nc.sync.dma_start(wT_sb[:], wT_v[:])
            idn = cpool.tile([128, 128], BF16)
            nc.sync.dma_start(idn[:], idn_ext[:])
            wp_sb = cpool.tile([128, KT, C], BF16)
            bias_sb = cpool.tile([1, C], F32)
            nc.sync.dma_start(bias_sb[:], bias_ext[:])
            bias_bc2 = cpool.tile([128, C], F32)
            nc.gpsimd.partition_broadcast(bias_bc2[:], bias_sb[:])
            # ones row: lhsT for the PE partition-broadcast
            # (rb = ones^T @ rcp)
            ones1 = cpool.tile([1, 64], BF16)
            nc.gpsimd.memset(ones1[:], 1.0)

            qT_sb = rpool.tile([128, NT], BF16)
            kT_sb = rpool.tile([128, NT], BF16)
            v_sb = rpool.tile([128, NT // 128, 130], BF16)
            nc.gpsimd.memset(v_sb[:, :, 64], 1.0)
            nc.gpsimd.memset(v_sb[:, :, 129], 1.0)
            stage = rpool.tile([65, 2 * NU, QC], F32)

            def qkv_groups(vpool, apsum, tpsum, x_tiles, bat, psum_tag):
                for nch2 in range(N // XCH):
                    x_t = x_tiles[bat * (N // XCH) + nch2]
                    for hw in range(XCH // QC):
                        ncol = bat * N + nch2 * XCH + hw * QC
                        for ft in range(3):
                            def emit(ncol=ncol, hw=hw, ft=ft, x_t=x_t):
                                xs = x_t[:, :, hw * QC:(hw + 1) * QC]
                                ps = apsum.tile([128, QC], F32, tag=psum_tag,
                                                name=f"qkv_{ncol}_{ft}")
                                for kt in range(KT):
                                    nc.tensor.matmul(
                                        ps[:],
                                        wT_sb[:, kt, ft * 128:(ft + 1) * 128],
                                        xs[:, kt, :],
                                        start=(kt == 0), stop=(kt == KT - 1))
                                if ft == 0:
                                    nc.vector.tensor_copy(
                                        out=qT_sb[:, ncol:ncol + QC],
                                        in_=ps[:])
                                elif ft == 1:
                                    nc.vector.tensor_copy(
                                        out=kT_sb[:, ncol:ncol + QC],
                                        in_=ps[:])
                                else:
                                    vt = vpool.tile([128, QC], BF16, tag="vt",
                                                    name=f"vt_{ncol}")
                                    nc.vector.tensor_copy(out=vt[:],
                                                          in_=ps[:])
                                    mtg0 = ncol // 128
                                    trp = tpsum.tile(
                                        [128, 4, 128], BF16, tag="tr",
                                        name=f"tr_{mtg0}")
                                    for t in range(4):
                                        nc.tensor.transpose(
                                            trp[:, t, :],
                                            vt[:, t * 128:(t + 1) * 128],
                                            idn[:])
                                    nc.vector.tensor_copy(
                                        out=v_sb[:, mtg0:mtg0 + 4, 0:64],
                                        in_=trp[:, :, 0:64])
                                    nc.vector.tensor_copy(
                                        out=v_sb[:, mtg0:mtg0 + 4, 65:129],
                                        in_=trp[:, :, 64:128])
                            yield emit

            def ag_closure(u, o_n2):
                def emit():
                    nc.sync.dma_start(ag_in[u][:], o_n2[:])
                    nc.gpsimd.collective_compute(
                        "AllGather",
                        mybir.AluOpType.bypass,
                        replica_groups=[list(range(NCORES))],
                        ins=[ag_in[u][:]],
                        outs=[ag_out[u][:]],
                    )
                return emit

            pair_tiles = {}

            def c_load_closure(u, lhspool):
                """Load my 64-token slice of unit u's gathered channels
                into half of the pair tile. Issued from gpsimd so the DMAs
                queue naturally behind the AllGather completions."""
                ag_v = ag_out[u].rearrange("(s p) q -> p s q", p=128)

                def emit():
                    pi = u // 2
                    if u % 2 == 0:
                        pair_tiles[pi] = lhspool.tile(
                            [128, NCORES, 128], BF16, tag="lhs",
                            name=f"lhs_{pi}")
                    lhs = pair_tiles[pi]
                    off = (u % 2) * 64
                    for s in range(NCORES):
                        nc.sync.dma_start(lhs[:, s, off:off + 64],
                                          ag_v[:, s, bass.ts(rank, 64)])
                return emit

            def c_mm_closure(pi, cpsum, postpool):
                """Out-projection for the 128 tokens of unit pair pi
                (M=128 full PE) + bias + fp32 out rows."""
                def emit():
                    lhs = pair_tiles[pi]
                    ob = postpool.tile([128, C], F32, tag="ob",
                                       name=f"ob_{pi}")
                    for half in range(2):
                        pc = cpsum.tile([128, QC], F32, tag="c",
                                        name=f"c_{pi}_{half}")
                        for s in range(NCORES):
                            nc.tensor.matmul(
                                pc[:],
                                lhs[:, s, :],
                                wp_sb[:, s, half * QC:(half + 1) * QC],
                                start=(s == 0), stop=(s == NCORES - 1))
                        nc.vector.tensor_tensor(
                            ob[:, half * QC:(half + 1) * QC], pc[:],
                            bias_bc2[:, half * QC:(half + 1) * QC],
                            mybir.AluOpType.add)
                    nc.sync.dma_start(
                        out_ext[pi * 128:(pi + 1) * 128, :], ob[:])
                return emit

            def norm_closure(u, o_n2, rcpb, rbps):
                """PE partition-broadcast of 1/den + normalize multiply +
                the AllGather. Drained mid-next-unit so the PE/DVE never
                wait on the reciprocal chain latency."""
                def emit():
                    for h in range(2):
                        rb = rbps.tile([64, QC], F32, tag="rb",
                                       name=f"rb_{u}_{h}")
                        nc.tensor.matmul(rb[:], ones1[:],
                                         rcpb[h][:],
                                         start=True, stop=True)
                        nc.vector.tensor_tensor(
                            o_n2[h * 64:(h + 1) * 64, :],
                            stage[0:64, u * 2 + h, :],
                            rb[:], mybir.AluOpType.mult)
                    nc.sync.dma_start(ag_in[u][:], o_n2[:])
                    nc.gpsimd.collective_compute(
                        "AllGather",
                        mybir.AluOpType.bypass,
                        replica_groups=[list(range(NCORES))],
                        ins=[ag_in[u][:]],
                        outs=[ag_out[u][:]],
                    )
                return emit

            def attn_phase(spsum, opsum, cpsum, rbps, ppool, denpool,
                           onpool, lhspool, postpool, bat, pend_norm,
                           pend_cl, pend_cm):
                for uu in range(N // QC):
                    u = bat * (N // QC) + uu
                    qcol = u * QC
                    units = [(h, mt) for mt in range(NMT) for h in range(2)]
                    o_cur = {}
                    o_n2 = onpool.tile([128, QC], BF16, tag="on",
                                       name=f"on_{u}")
                    den2 = denpool.tile([33, QC], F32, tag="den2",
                                        name=f"den2_{u}")
                    rcp2 = denpool.tile([33, QC], F32, tag="rcp2",
                                        name=f"rcp2_{u}")
                    rcpb = [denpool.tile([1, QC], BF16, tag=f"rcpb{h}",
                                         name=f"rcpb{h}_{u}")
                            for h in range(2)]
                    heads_done = [0]
                    for g0 in range(0, len(units), GRP):
                        g = units[g0:g0 + GRP]
                        s_t = spsum.tile([128, GRP, QC], F32, tag="s",
                                         name=f"s_{u}_{g0}")
                        for ui, (h, mt) in enumerate(g):
                            if mt == 0 and h not in o_cur:
                                o_cur[h] = opsum.tile(
                                    [65, QC], F32, tag=f"o{h}",
                                    name=f"o_ps{h}_{u}")
                            nc.tensor.matmul(
                                s_t[:, ui, :],
                                kT_sb[h * 64:(h + 1) * 64,
                                      bat * N + mt * 128:
                                      bat * N + (mt + 1) * 128],
                                qT_sb[h * 64:(h + 1) * 64, qcol:qcol + QC],
                                start=True, stop=True)
                        p_t = ppool.tile([128, GRP, QC], BF16, tag="p",
                                         name=f"p_{u}_{g0}")
                        nc.scalar.activation(p_t[:, 0:len(g), :],
                                             s_t[:, 0:len(g), :],
                                             EXP, scale=SCALE)
                        for ui, (h, mt) in enumerate(g):
                            nc.tensor.matmul(
                                o_cur[h][:],
                                v_sb[:, bat * NMT + mt, h * 65:(h + 1) * 65],
                                p_t[:, ui, :],
                                start=(mt == 0), stop=(mt == NMT - 1))
                            if mt == NMT - 1:
                                o_ps = o_cur.pop(h)
                                nc.vector.tensor_copy(
                                    out=stage[:, u * 2 + h, :],
                                    in_=o_ps[0:65, :])
                                # den row -> partition 32*h of den2
                                nc.vector.tensor_copy(
                                    out=den2[32 * h:32 * h + 1, :],
                                    in_=o_ps[64:65, :])
                                heads_done[0] += 1
                                if heads_done[0] == 2:
                                    # one reciprocal serves both heads
                                    nc.vector.reciprocal(rcp2[:], den2[:])
                                    for hh in range(2):
                                        nc.vector.tensor_copy(
                                            out=rcpb[hh][:],
                                            in_=rcp2[32 * hh:
                                                     32 * hh + 1, :])
                                    pend_norm.append(
                                        norm_closure(u, o_n2, rcpb, rbps))
                                    pend_cl.append(
                                        c_load_closure(u, lhspool))
                                    if u % 2 == 1:
                                        pend_cm.append(
                                            c_mm_closure(u // 2, cpsum,
                                                         postpool))
                        # norm tail drains mid-unit (reciprocal latency
                        # settled); loads ~3 units behind (AG#u completed);
                        # matmuls one pair behind the loads
                        if pend_norm and g0 // GRP >= 9:
                            pend_norm.pop(0)()
                        if len(pend_cl) > 2:
                            pend_cl.pop(0)()
                        if len(pend_cm) > 1:
                            pend_cm.pop(0)()

            with (
                tc.tile_pool(name="xchunk", bufs=1) as xpool,
                tc.tile_pool(name="vtmp", bufs=2) as vpool,
                tc.tile_pool(name="pexp", bufs=4) as ppool,
                tc.tile_pool(name="denp", bufs=4) as denpool,
                tc.tile_pool(name="onrm", bufs=3) as onpool,
                tc.tile_pool(name="lhsp", bufs=2) as lhspool,
                tc.tile_pool(name="postp", bufs=2) as postpool,
                tc.tile_pool(name="cps", bufs=1, space="PSUM") as cpsum,
                tc.tile_pool(name="rbps", bufs=1, space="PSUM") as rbps,
            ):
                x_tiles = []
                for nch in range(NT // XCH):
                    x_t = xpool.tile([128, KT, XCH], BF16, tag=f"x{nch}",
                                     name=f"x_{nch}")
                    x_tiles.append(x_t)
                for kt in range(KT):
                    nc.sync.dma_start(x_tiles[0][:, kt, :],
                                      xT_v[:, kt, 0:XCH])
                # remaining x chunks, then big weights (wp is not needed
                # until the first C pair)
                for nch in range(1, NT // XCH):
                    nc.sync.dma_start(
                        x_tiles[nch][:],
                        xT_v[:, :, nch * XCH:(nch + 1) * XCH])
                nc.sync.dma_start(wp_sb[:], wpT_v[:])

                pend_norm = []
                pend_cl = []
                pend_cm = []
                for bat in range(B):
                    with (
                        tc.tile_pool(name=f"qkvps{bat}", bufs=2,
                                     space="PSUM") as apsum,
                        tc.tile_pool(name=f"trps{bat}", bufs=2,
                                     space="PSUM") as tpsum,
                    ):
                        for gi, emit in enumerate(qkv_groups(
                                vpool, apsum, tpsum, x_tiles, bat,
                                f"a{bat}")):
                            emit()
                            if gi >= 2 and pend_norm:
                                pend_norm.pop(0)()
                    with (
                        tc.tile_pool(name=f"sps{bat}", bufs=2,
                                     space="PSUM") as spsum,
                        tc.tile_pool(name=f"ops{bat}", bufs=1,
                                     space="PSUM") as opsum,
                    ):
                        attn_phase(spsum, opsum, cpsum, rbps, ppool,
                                   denpool, onpool, lhspool, postpool, bat,
                                   pend_norm, pend_cl, pend_cm)
                        if bat == B - 1:
                            for f in pend_norm:
                                f()
                            pend_norm.clear()
                            for f in pend_cl:
                                f()
                            pend_cl.clear()
                            for f in pend_cm:
                                f()
                            pend_cm.clear()
    nc.compile()
    return nc


def kernel(x, w_qkv, w_proj, b_proj):
    global _NC, LAST_EXEC_NS
    if _NC is None:
        _NC = _build()
    x = np.asarray(x, dtype=np.float32)
    w_qkv = np.asarray(w_qkv, dtype=np.float32)
    w_proj = np.asarray(w_proj, dtype=np.float32)
    b_proj = np.asarray(b_proj, dtype=np.float32)

    import ml_dtypes
    xT = np.ascontiguousarray(x.reshape(NT, C).T).astype(ml_dtypes.bfloat16)
    wpT = np.ascontiguousarray(w_proj.T).astype(ml_dtypes.bfloat16)
    bias = np.ascontiguousarray(b_proj.reshape(1, C))
    idn = np.eye(128, dtype=ml_dtypes.bfloat16)
    in_maps = []
    for c in range(NCORES):
        blk = slice(128 * c, 128 * (c + 1))
        wT = np.ascontiguousarray(
            np.concatenate([w_qkv[0:C][blk], w_qkv[C:2 * C][blk],
                            w_qkv[2 * C:3 * C][blk]], axis=0).T).astype(
                ml_dtypes.bfloat16)
        in_maps.append({"xT": xT, "wT": wT, "wpT": wpT, "bias": bias,
                        "idn": idn})

    if TRACE:
        _install_ntff_hook()
    res = run_bass_kernel_spmd(_NC, in_maps, core_ids=list(range(NCORES)),
                               trace=TRACE)
    LAST_EXEC_NS = res.exec_time_ns
    # core c's rows are (u, 64): global token u*512 + c*64 + i
    arr = np.stack([res.results[i]["out"] for i in range(NCORES)])
    out = arr.reshape(NCORES, NU, 64, C).transpose(1, 0, 2, 3)
    return np.ascontiguousarray(
        out.reshape(B, N, C).astype(np.float32))


# revision 29
# speedup vs baseline: 1.0087x; 1.0087x over previous
"""Distributed multi-head attention for 8 TRN2 NeuronCores — v4 (AllGather).

Problem: x[2,2048,1024] -> QKV proj (w_qkv[3072,1024]) -> 16-head SDPA ->
out proj (w_proj[1024,1024] + b_proj) -> [2,2048,1024].

Sharding: 2 heads per core (head-parallel; both batches on every core).
  Phase A: qT/kT [128, 4096] and V-natural from x @ w_qkv_shard.T.
  Phase B: transposed-score attention per (batch, 512-query chunk) u:
           S^T = kT.T @ qT (row-tiled K=64 pairs), P = exp(S*scale),
           O^T_ext[65,n] = [V|1].T @ P^T accumulated in PSUM (row 64 =
           denominator). Normalized o_n2[128ch, 512tok] per unit.
  Exchange: per-unit AllGather of o_n2 (128KB in, 1MB out, Shared) —
           8 small AGs overlap attention compute.
  Out-proj at destination: each core computes out rows for ITS 64-token
           slice of unit u (cc_rank dynamic slice): [64,1024] over all
           1024 channels, + bias -> fp32 out rows. Host reorders rows.
"""
import sys, os, types
import numpy as np

if "/opt/trn_rl_repo" not in sys.path and os.path.isdir("/opt/trn_rl_repo"):
    sys.path.append("/opt/trn_rl_repo")

import concourse.bass as bass
import concourse.mybir as mybir
import concourse.tile as tile
from concourse import bacc
from concourse.bass_utils import run_bass_kernel_spmd

F32 = mybir.dt.float32
F16 = mybir.dt.float16
BF16 = mybir.dt.bfloat16
EXP = mybir.ActivationFunctionType.Exp

NCORES = 8
B, N, C, H, D = 2, 2048, 1024, 16, 64
NT = B * N
KT = C // 128
QC = 512
NU = NT // QC
NMT = N // 128
SCALE = 1.0 / 8.0
GRP = 2
XCH = 1024

TRACE = False
LAST_EXEC_NS = None

_NC = None


def _install_ntff_hook():
    if "antenv.axon_hooks" in sys.modules:
        return
    try:
        import antenv
        from trn_agent_boot.trn_boot import _ntff_profile_via_ctypes
        mod = types.ModuleType("antenv.axon_hooks")
        _hook = [None]
        mod.set_axon_ntff_profile_hook = lambda h: _hook.__setitem__(0, h)
        mod.get_axon_ntff_profile_hook = lambda: _hook[0]
        sys.modules["antenv.axon_hooks"] = mod
        antenv.axon_hooks = mod
        mod.set_axon_ntff_profile_hook(
            _ntff_profile_via_ctypes("/opt/axon/libaxon_pjrt.so"))
    except Exception:
        pass


def _build():
    nc = bacc.Bacc("TRN2", target_bir_lowering=False, debug=False,
                   num_devices=NCORES)
    xT_ext = nc.dram_tensor("xT", [C, NT], BF16, kind="ExternalInput").ap()
    wT_ext = nc.dram_tensor("wT", [C, 384], BF16, kind="ExternalInput").ap()
    wpT_ext = nc.dram_tensor("wpT", [C, C], BF16, kind="ExternalInput").ap()
    bias_ext = nc.dram_tensor("bias", [1, C], F32, kind="ExternalInput").ap()
    idn_ext = nc.dram_tensor("idn", [128, 128], BF16, kind="ExternalInput").ap()
    out_ext = nc.dram_tensor("out", [NT // NCORES, C], F16,
                             kind="ExternalOutput").ap()
    ag_in = [nc.dram_tensor(f"ag_in{u}", [128, QC], BF16).ap()
             for u in range(NU)]
    ag_out = [nc.dram_tensor(f"ag_out{u}", [NCORES * 128, QC], BF16,
                             addr_space="Shared").ap()
              for u in range(NU)]

    xT_v = xT_ext.rearrange("(kt p) n -> p kt n", p=128)
    wT_v = wT_ext.rearrange("(kt p) f -> p kt f", p=128)
    wpT_v = wpT_ext.rearrange("(kt p) f -> p kt f", p=128)

    with tile.TileContext(nc) as tc:
        rank = nc.sync.cc_rank([list(range(NCORES))])
        with (
            tc.tile_pool(name="const", bufs=1) as cpool,
            tc.tile_pool(name="resid", bufs=1) as rpool,
        ):
            wT_sb = cpool.tile([128, KT, 384], BF16)
            nc.sync.dma_start(wT_sb[:], wT_v[:])
            idn = cpool.tile([128, 128], BF16)
            nc.sync.dma_start(idn[:], idn_ext[:])
            wp_sb = cpool.tile([128, KT, C], BF16)
            bias_sb = cpool.tile([1, C], F32)
            nc.sync.dma_start(bias_sb[:], bias_ext[:])
            bias_bc2 = cpool.tile([128, C], F32)
            nc.gpsimd.partition_broadcast(bias_bc2[:], bias_sb[:])
            # ones row: lhsT for the PE partition-broadcast
            # (rb = ones^T @ rcp)
            ones1 = cpool.tile([1, 64], BF16)
            nc.gpsimd.memset(ones1[:], 1.0)

            qT_sb = rpool.tile([128, NT], BF16)
            kT_sb = rpool.tile([128, NT], BF16)
            v_sb = rpool.tile([128, NT // 128, 130], BF16)
            nc.gpsimd.memset(v_sb[:, :, 64], 1.0)
            nc.gpsimd.memset(v_sb[:, :, 129], 1.0)
            stage = rpool.tile([65, 2 * NU, QC], F32)

            def qkv_groups(vpool, apsum, tpsum, x_tiles, bat, psum_tag):
                for nch2 in range(N // XCH):
                    x_t = x_tiles[bat * (N // XCH) + nch2]
                    for hw in range(XCH // QC):
                        ncol = bat * N + nch2 * XCH + hw * QC
                        for ft in range(3):
                            def emit(ncol=ncol, hw=hw, ft=ft, x_t=x_t):
                                xs = x_t[:, :, hw * QC:(hw + 1) * QC]
                                ps = apsum.tile([128, QC], F32, tag=psum_tag,
                                                name=f"qkv_{ncol}_{ft}")
                                for kt in range(KT):
                                    nc.tensor.matmul(
                                        ps[:],
                                        wT_sb[:, kt, ft * 128:(ft + 1) * 128],
                                        xs[:, kt, :],
                                        start=(kt == 0), stop=(kt == KT - 1))
                                if ft == 0:
                                    nc.vector.tensor_copy(
                                        out=qT_sb[:, ncol:ncol + QC],
                                        in_=ps[:])
                                elif ft == 1:
                                    nc.vector.tensor_copy(
                                        out=kT_sb[:, ncol:ncol + QC],
                                        in_=ps[:])
                                else:
                                    vt = vpool.tile([128, QC], BF16, tag="vt",
                                                    name=f"vt_{ncol}")
                                    nc.vector.tensor_copy(out=vt[:],
                                                          in_=ps[:])
                                    mtg0 = ncol // 128
                                    trp = tpsum.tile(
                                        [128, 4, 128], BF16, tag="tr",
                                        name=f"tr_{mtg0}")
                                    for t in range(4):
                                        nc.tensor.transpose(
                                            trp[:, t, :],
                                            vt[:, t * 128:(t + 1) * 128],
                                            idn[:])
                                    nc.vector.tensor_copy(
                                        out=v_sb[:, mtg0:mtg0 + 4, 0:64],
                                        in_=trp[:, :, 0:64])
                                    nc.vector.tensor_copy(
                                        out=v_sb[:, mtg0:mtg0 + 4, 65:129],
                                        in_=trp[:, :, 64:128])
                            yield emit

            def ag_closure(u, o_n2):
                def emit():
                    nc.sync.dma_start(ag_in[u][:], o_n2[:])
                    nc.gpsimd.collective_compute(
                        "AllGather",
                        mybir.AluOpType.bypass,
                        replica_groups=[list(range(NCORES))],
                        ins=[ag_in[u][:]],
                        outs=[ag_out[u][:]],
                    )
                return emit

            pair_tiles = {}

            def c_load_closure(u, lhspool):
                """Load my 64-token slice of unit u's gathered channels
                into half of the pair tile. Issued from gpsimd so the DMAs
                queue naturally behind the AllGather completions."""
                ag_v = ag_out[u].rearrange("(s p) q -> p s q", p=128)

                def emit():
                    pi = u // 2
                    if u % 2 == 0:
                        pair_tiles[pi] = lhspool.tile(
                            [128, NCORES, 128], BF16, tag="lhs",
                            name=f"lhs_{pi}")
                    lhs = pair_tiles[pi]
                    off = (u % 2) * 64
                    for s in range(NCORES):
                        nc.sync.dma_start(lhs[:, s, off:off + 64],
                                          ag_v[:, s, bass.ts(rank, 64)])
                return emit

            def c_mm_closure(pi, cpsum, postpool):
                """Out-projection for the 128 tokens of unit pair pi
                (M=128 full PE) + bias + fp32 out rows."""
                def emit():
                    lhs = pair_tiles[pi]
                    ob = postpool.tile([128, C], F16, tag="ob",
                                       name=f"ob_{pi}")
                    for half in range(2):
                        pc = cpsum.tile([128, QC], F32, tag="c",
                                        name=f"c_{pi}_{half}")
                        for s in range(NCORES):
                            nc.tensor.matmul(
                                pc[:],
                                lhs[:, s, :],
                                wp_sb[:, s, half * QC:(half + 1) * QC],
                                start=(s == 0), stop=(s == NCORES - 1))
                        nc.vector.tensor_tensor(
                            ob[:, half * QC:(half + 1) * QC], pc[:],
                            bias_bc2[:, half * QC:(half + 1) * QC],
                            mybir.AluOpType.add)
                    nc.sync.dma_start(
                        out_ext[pi * 128:(pi + 1) * 128, :], ob[:])
                return emit

            def norm_closure(u, o_n2, rcpb, rbps):
                """PE partition-broadcast of 1/den + normalize multiply +
                the AllGather. Drained mid-next-unit so the PE/DVE never
                wait on the reciprocal chain latency."""
                def emit():
                    rb2 = rbps.tile([128, QC], F32, tag="rb",
                                    name=f"rb_{u}")
                    for h in range(2):
                        nc.tensor.matmul(rb2[h * 64:(h + 1) * 64, :],
                                         ones1[:], rcpb[h][:],
                                         start=True, stop=True)
                    for h in range(2):
                        nc.vector.tensor_tensor(
                            o_n2[h * 64:(h + 1) * 64, :],
                            stage[0:64, u * 2 + h, :],
                            rb2[h * 64:(h + 1) * 64, :],
                            mybir.AluOpType.mult)
                    nc.sync.dma_start(ag_in[u][:], o_n2[:])
                    nc.gpsimd.collective_compute(
                        "AllGather",
                        mybir.AluOpType.bypass,
                        replica_groups=[list(range(NCORES))],
                        ins=[ag_in[u][:]],
                        outs=[ag_out[u][:]],
                    )
                return emit

            def attn_phase(spsum, opsum, cpsum, rbps, ppool, denpool,
                           onpool, lhspool, postpool, bat, pend_norm,
                           pend_cl, pend_cm):
                for uu in range(N // QC):
                    u = bat * (N // QC) + uu
                    qcol = u * QC
                    units = [(h, mt) for mt in range(NMT) for h in range(2)]
                    o_cur = {}
                    o_n2 = onpool.tile([128, QC], BF16, tag="on",
                                       name=f"on_{u}")
                    den2 = denpool.tile([33, QC], F32, tag="den2",
                                        name=f"den2_{u}")
                    rcp2 = denpool.tile([33, QC], F32, tag="rcp2",
                                        name=f"rcp2_{u}")
                    rcpb = [denpool.tile([1, QC], BF16, tag=f"rcpb{h}",
                                         name=f"rcpb{h}_{u}")
                            for h in range(2)]
                    heads_done = [0]
                    for g0 in range(0, len(units), GRP):
                        g = units[g0:g0 + GRP]
                        s_t = spsum.tile([128, GRP, QC], F32, tag="s",
                                         name=f"s_{u}_{g0}")
                        for ui, (h, mt) in enumerate(g):
                            if mt == 0 and h not in o_cur:
                                o_cur[h] = opsum.tile(
                                    [65, QC], F32, tag=f"o{h}",
                                    name=f"o_ps{h}_{u}")
                            nc.tensor.matmul(
                                s_t[:, ui, :],
                                kT_sb[h * 64:(h + 1) * 64,
                                      bat * N + mt * 128:
                                      bat * N + (mt + 1) * 128],
                                qT_sb[h * 64:(h + 1) * 64, qcol:qcol + QC],
                                start=True, stop=True)
                        p_t = ppool.tile([128, GRP, QC], BF16, tag="p",
                                         name=f"p_{u}_{g0}")
                        nc.scalar.activation(p_t[:, 0:len(g), :],
                                             s_t[:, 0:len(g), :],
                                             EXP, scale=SCALE)
                        for ui, (h, mt) in enumerate(g):
                            nc.tensor.matmul(
                                o_cur[h][:],
                                v_sb[:, bat * NMT + mt, h * 65:(h + 1) * 65],
                                p_t[:, ui, :],
                                start=(mt == 0), stop=(mt == NMT - 1))
                            if mt == NMT - 1:
                                o_ps = o_cur.pop(h)
                                nc.vector.tensor_copy(
                                    out=stage[:, u * 2 + h, :],
                                    in_=o_ps[0:65, :])
                                # den row -> partition 32*h of den2
                                nc.vector.tensor_copy(
                                    out=den2[32 * h:32 * h + 1, :],
                                    in_=o_ps[64:65, :])
                                heads_done[0] += 1
                                if heads_done[0] == 2:
                                    # one reciprocal serves both heads
                                    nc.vector.reciprocal(rcp2[:], den2[:])
                                    for hh in range(2):
                                        nc.vector.tensor_copy(
                                            out=rcpb[hh][:],
                                            in_=rcp2[32 * hh:
                                                     32 * hh + 1, :])
                                    pend_norm.append(
                                        norm_closure(u, o_n2, rcpb, rbps))
                                    pend_cl.append(
                                        c_load_closure(u, lhspool))
                                    if u % 2 == 1:
                                        pend_cm.append(
                                            c_mm_closure(u // 2, cpsum,
                                                         postpool))
                        # norm tail drains mid-unit (reciprocal latency
                        # settled); loads ~3 units behind (AG#u completed);
                        # matmuls one pair behind the loads
                        if pend_norm and g0 // GRP >= 9:
                            pend_norm.pop(0)()
                        if len(pend_cl) > 2:
                            pend_cl.pop(0)()
                        if len(pend_cm) > 2:
                            pend_cm.pop(0)()

            with (
                tc.tile_pool(name="xchunk", bufs=1) as xpool,
                tc.tile_pool(name="vtmp", bufs=2) as vpool,
                tc.tile_pool(name="pexp", bufs=4) as ppool,
                tc.tile_pool(name="denp", bufs=4) as denpool,
                tc.tile_pool(name="onrm", bufs=3) as onpool,
                tc.tile_pool(name="lhsp", bufs=2) as lhspool,
                tc.tile_pool(name="postp", bufs=2) as postpool,
                tc.tile_pool(name="cps", bufs=1, space="PSUM") as cpsum,
                tc.tile_pool(name="rbps", bufs=1, space="PSUM") as rbps,
            ):
                x_tiles = []
                for nch in range(NT // XCH):
                    x_t = xpool.tile([128, KT, XCH], BF16, tag=f"x{nch}",
                                     name=f"x_{nch}")
                    x_tiles.append(x_t)
                for kt in range(KT):
                    nc.sync.dma_start(x_tiles[0][:, kt, :],
                                      xT_v[:, kt, 0:XCH])
                # remaining x chunks, then big weights (wp is not needed
                # until the first C pair)
                for nch in range(1, NT // XCH):
                    nc.sync.dma_start(
                        x_tiles[nch][:],
                        xT_v[:, :, nch * XCH:(nch + 1) * XCH])
                nc.sync.dma_start(wp_sb[:], wpT_v[:])

                pend_norm = []
                pend_cl = []
                pend_cm = []
                for bat in range(B):
                    with (
                        tc.tile_pool(name=f"qkvps{bat}", bufs=2,
                                     space="PSUM") as apsum,
                        tc.tile_pool(name=f"trps{bat}", bufs=2,
                                     space="PSUM") as tpsum,
                    ):
                        for gi, emit in enumerate(qkv_groups(
                                vpool, apsum, tpsum, x_tiles, bat,
                                f"a{bat}")):
                            emit()
                            if gi >= 2 and pend_norm:
                                pend_norm.pop(0)()
                    with (
                        tc.tile_pool(name=f"sps{bat}", bufs=2,
                                     space="PSUM") as spsum,
                        tc.tile_pool(name=f"ops{bat}", bufs=1,
                                     space="PSUM") as opsum,
                    ):
                        attn_phase(spsum, opsum, cpsum, rbps, ppool,
                                   denpool, onpool, lhspool, postpool, bat,
                                   pend_norm, pend_cl, pend_cm)
                        if bat == B - 1:
                            for f in pend_norm:
                                f()
                            pend_norm.clear()
                            for f in pend_cl:
                                f()
                            pend_cl.clear()
                            for f in pend_cm:
                                f()
                            pend_cm.clear()
    nc.compile()
    return nc


def kernel(x, w_qkv, w_proj, b_proj):
    global _NC, LAST_EXEC_NS
    if _NC is None:
        _NC = _build()
    x = np.asarray(x, dtype=np.float32)
    w_qkv = np.asarray(w_qkv, dtype=np.float32)
    w_proj = np.asarray(w_proj, dtype=np.float32)
    b_proj = np.asarray(b_proj, dtype=np.float32)

    import ml_dtypes
    xT = np.ascontiguousarray(x.reshape(NT, C).T).astype(ml_dtypes.bfloat16)
    wpT = np.ascontiguousarray(w_proj.T).astype(ml_dtypes.bfloat16)
    bias = np.ascontiguousarray(b_proj.reshape(1, C))
    idn = np.eye(128, dtype=ml_dtypes.bfloat16)
    in_maps = []
    for c in range(NCORES):
        blk = slice(128 * c, 128 * (c + 1))
        wT = np.ascontiguousarray(
            np.concatenate([w_qkv[0:C][blk], w_qkv[C:2 * C][blk],
                            w_qkv[2 * C:3 * C][blk]], axis=0).T).astype(
                ml_dtypes.bfloat16)
        in_maps.append({"xT": xT, "wT": wT, "wpT": wpT, "bias": bias,
                        "idn": idn})

    if TRACE:
        _install_ntff_hook()
    res = run_bass_kernel_spmd(_NC, in_maps, core_ids=list(range(NCORES)),
                               trace=TRACE)
    LAST_EXEC_NS = res.exec_time_ns
    # core c's rows are (u, 64): global token u*512 + c*64 + i
    arr = np.stack([res.results[i]["out"] for i in range(NCORES)])
    out = arr.reshape(NCORES, NU, 64, C).transpose(1, 0, 2, 3)
    return np.ascontiguousarray(
        out.reshape(B, N, C).astype(np.float32))
